# revision 40
# baseline (speedup 1.0000x reference)
"""Trainium2 Bass kernel for nn_AttLoc (location-aware attention).

Math (per batch b):
  pre_enc[t,a] = enc[t,:] @ W_enc.T          (T=1600, E=512, A=512)
  att_conv[t,c]= conv1d(att_prev, conv_w)    (C=10, K=201, SAME pad 100)
  att2[t,a]    = att_conv[t,:] @ W_att.T
  dec[a]       = dec_z @ W_dec.T + b_enc
  e[t]         = gvec . tanh(pre_enc + att2 + dec)
  w            = softmax(2*e)                (gvec_b shift cancels in softmax)
  c[d]         = sum_t w[t] * enc[t,d]

Sharding: data-parallel over batch B=32 across 8 cores (4 batches/core),
weights replicated. Host pre-transposes operands into PE layouts
(contraction dim on partitions) and builds the conv Toeplitz (im2col).

Device schedule per core (z in layout: att-dim A on partitions, t free):
  - conv stage1 on PE produces attc[c,t] from a host-built Toeplitz; an
    extra Toeplitz ones-row + conv channel makes attc row 10 == 1.0, which
    carries the (dec_z @ W_dec.T + b_enc) bias as the 11th contraction row
    of the stage2 matmul (bias values are patched into the stationary
    operand on device).
  - main matmul: 4 K-passes of W_encT.T @ encT into PSUM + the K=11 att
    matmul; tanh on ACT over two PSUM banks at a time (no ACT bias needed
    since the bias rode the matmul).
  - e[t] via PE matvec (gvec stationary); softmax runs per batch with
    chunked running max, so batch b+1's matmuls overlap batch b's
    softmax/context phase; 1/sum(exp) is folded into the normalized bf16
    w row.
  - context c[d] on the PE: the normalized w row is transposed into
    t-on-partitions via the DMA XBAR (13 [32,128] tiles), then 13
    accumulating [K=128, M=1, N=512] matvecs against a second, natural
    layout enc stream (encN).
"""

import os
import numpy as np

B, T, EPROJS, DUNITS, ATT_DIM = 32, 1600, 512, 1024, 512
ACONV_CHANS, ACONV_FILTS = 10, 100
KSIZE = 2 * ACONV_FILTS + 1  # 201
SCALING = 2.0

NCORES = 8
NB = B // NCORES              # 4 batches per core
NKP = EPROJS // 128           # 4 K-passes over eprojs
NMT = ATT_DIM // 128          # 4 M-tiles over att_dim
NKD = DUNITS // 128           # 8 K-passes over dunits
CHUNKS = [(0, 512), (512, 512), (1024, 512), (1536, 64)]

_DT_MM = os.environ.get("KERNEL_DT", "bf16")
# engine for the context multiply: pool (gpsimd) or dve
_CMUL = os.environ.get("KERNEL_CMUL", "dve")
# benchmark: repeat the steady-state body N times inside the kernel
_LOOP = int(os.environ.get("KERNEL_LOOP", "1"))

_cache = {}


def _np_dt():
    if _DT_MM == "fp32":
        return np.float32
    import ml_dtypes
    return ml_dtypes.bfloat16


def _build_nc():
    import concourse.bacc as bacc
    import concourse.mybir as mybir
    import concourse.tile as tile

    f32 = mybir.dt.float32
    dt_mm = f32 if _DT_MM == "fp32" else mybir.dt.bfloat16

    nc = bacc.Bacc(None, target_bir_lowering=False)

    # -------- per-core DRAM tensors --------
    encT_d = nc.dram_tensor("encT", [NB, NKP, 128, T], dt_mm, kind="ExternalInput")
    encN_d = nc.dram_tensor("encN", [NB, 13, 128, ATT_DIM], dt_mm, kind="ExternalInput")
    toep_d = nc.dram_tensor("toep", [NB, 2, 128, T], dt_mm, kind="ExternalInput")
    wencT_d = nc.dram_tensor("wencT", [NKP, 128, NMT * 128], dt_mm, kind="ExternalInput")
    watt4_d = nc.dram_tensor("watt4", [ACONV_CHANS + 1, ATT_DIM], dt_mm, kind="ExternalInput")
    convwT_d = nc.dram_tensor("convwT", [2, 128, 32], dt_mm, kind="ExternalInput")
    wdecT_d = nc.dram_tensor("wdecT", [NKD, 128, ATT_DIM], dt_mm, kind="ExternalInput")
    deczT_d = nc.dram_tensor("deczT", [NKD, 128, NB], dt_mm, kind="ExternalInput")
    bencT_d = nc.dram_tensor("bencT", [1, ATT_DIM], dt_mm, kind="ExternalInput")
    gvec_d = nc.dram_tensor("gvec", [128, NMT], dt_mm, kind="ExternalInput")
    ident_d = nc.dram_tensor("ident", [128, 128], dt_mm, kind="ExternalInput")

    c_out_d = nc.dram_tensor("c_out", [NB, ATT_DIM], f32, kind="ExternalOutput")
    w_out_d = nc.dram_tensor("w_out", [NB * T], f32, kind="ExternalOutput")

    NC11 = ACONV_CHANS + 1  # 11: conv channels + ones row

    with tile.TileContext(nc) as tc:
        with (
            tc.tile_pool(name="const", bufs=1) as constp,
            tc.tile_pool(name="enc", bufs=NB) as encp,
            tc.tile_pool(name="toep", bufs=2) as toepp,
            tc.tile_pool(name="work", bufs=2) as workp,
            tc.tile_pool(name="tanh", bufs=3) as tanhp,
            tc.tile_pool(name="small", bufs=1) as smallp,
            tc.tile_pool(name="psum_z", bufs=int(os.environ.get("KERNEL_ZB", "2")), space="PSUM") as zp,
            tc.tile_pool(name="psum_a", bufs=int(os.environ.get("KERNEL_AB", "2")), space="PSUM") as sp,
            tc.tile_pool(name="psum_e", bufs=int(os.environ.get("KERNEL_EB", "2")), space="PSUM") as ep,
        ):
            # ---- constants / weights (dec-chain weights first: the dec
            #      bias gates every stage2 matmul via the watt4 patch) ----
            deczT = constp.tile([128, NKD, NB], dt_mm)
            nc.sync.dma_start(deczT[:], deczT_d.rearrange("k p b -> p k b"))
            wdecT = constp.tile([128, NKD, ATT_DIM], dt_mm)
            nc.sync.dma_start(
                wdecT[:, :, 0:128],
                wdecT_d[:, :, 0:128].rearrange("k p m -> p k m"),
            )
            bencT = constp.tile([1, ATT_DIM], dt_mm)
            nc.sync.dma_start(bencT[:], bencT_d[:])
            for m in range(1, NMT):
                nc.sync.dma_start(
                    wdecT[:, :, m * 128:(m + 1) * 128],
                    wdecT_d[:, :, m * 128:(m + 1) * 128].rearrange(
                        "k p m -> p k m"),
                )
            ident = constp.tile([128, 128], dt_mm)
            nc.sync.dma_start(ident[:], ident_d[:])
            wencT = constp.tile([128, NKP, NMT * 128], dt_mm)
            nc.sync.dma_start(wencT[:], wencT_d.rearrange("k p m -> p k m"))
            convwT = constp.tile([128, 2, 32], dt_mm)
            nc.sync.dma_start(convwT[:], convwT_d.rearrange("s p c -> p s c"))
            gvec = constp.tile([128, NMT], dt_mm)
            nc.sync.dma_start(gvec[:], gvec_d[:])
            ones = constp.tile([1, 128], dt_mm)
            nc.vector.memset(ones[:], 1.0)
            # per-batch stationary for stage2 (wattT in 32-row groups; row
            # 32m+10 patched below with the dec bias for (m, b))
            watt4 = []
            for b in range(NB):
                wt = constp.tile([NC11, ATT_DIM], dt_mm, tag=f"watt4_{b}")
                nc.sync.dma_start(wt[:], watt4_d[:])
                watt4.append(wt)

            # ---- dec bias: dec_z @ W_dec.T + b_enc -> rows of watt4 ----
            def emit_dec():
                dec_ps = sp.tile([128, 512], mybir.dt.float32, tag="attc",
                                 name="dec_ps")
                for m in range(NMT):
                    cols = slice(m * NB, (m + 1) * NB)
                    for kp in range(NKD):
                        nc.tensor.matmul(
                            dec_ps[:, cols],
                            wdecT[:, kp, m * 128:(m + 1) * 128],
                            deczT[:, kp, :],
                            start=(kp == 0), stop=False,
                        )
                    nc.tensor.matmul(
                        dec_ps[:, cols],
                        bencT[0:1, m * 128:(m + 1) * 128],
                        ones[0:1, 0:NB],
                        start=False, stop=True,
                    )
                dec_sb = smallp.tile([128, NMT * NB], dt_mm, name="dec_sb")
                nc.scalar.copy(dec_sb[:], dec_ps[:, :NMT * NB])
                biasT_ps = sp.tile([NMT * NB, 128], dt_mm, tag="attc",
                                   name="biasT_ps")
                nc.tensor.transpose(biasT_ps[:], dec_sb[:], ident[:])
                biasT = smallp.tile([NMT * NB, 128], dt_mm, name="biasT")
                nc.scalar.copy(biasT[:], biasT_ps[:])
                for b in range(NB):
                    for m in range(NMT):
                        nc.sync.dma_start(
                            watt4[b][ACONV_CHANS:NC11, m * 128:(m + 1) * 128],
                            biasT[m * NB + b:m * NB + b + 1, :],
                        )

            # ---- persistent per-batch encT tiles (DMA'd with distance-1
            #      prefetch from inside the batch loop) ----
            enc_tiles = []
            encn_tiles = []
            for b in range(NB):
                et = encp.tile([128, NKP, T], dt_mm, tag="encT", name=f"encT{b}")
                enc_tiles.append(et)
                en = encp.tile([128, 13, ATT_DIM], dt_mm, tag="encN",
                               name=f"encN{b}")
                encn_tiles.append(en)

            e_stage = smallp.tile([1, NB * T], f32)
            w128 = smallp.tile([128, 13 * 128], dt_mm)
            emax_all = smallp.tile([1, NB], f32)
            nb2_all = smallp.tile([1, NB], f32)
            esum_all = smallp.tile([1, NB], f32)
            rinv_all = smallp.tile([1, NB], f32)
            emax4 = smallp.tile([1, NB * len(CHUNKS)], f32)
            wexp_st = smallp.tile([1, NB * T], dt_mm)

            tps = [None] * NB

            def emit_mains(b, chunks):
                if tps[b] is None:
                    tp = toepp.tile([128, 2, T], dt_mm, tag="toep", name="tp")
                    tps[b] = tp
                    for (t0, tw) in CHUNKS:
                        nc.sync.dma_start(
                            tp[:, :, t0:t0 + tw],
                            toep_d[b][:, :, t0:t0 + tw].rearrange(
                                "s p t -> p s t"),
                        )
                    nc.sync.dma_start(
                        encn_tiles[b][:],
                        encN_d[b].rearrange("c p e -> p c e"),
                    )
                    if b + 1 < NB:
                        for (t0, tw) in CHUNKS:
                            nc.sync.dma_start(
                                enc_tiles[b + 1][:, :, t0:t0 + tw],
                                encT_d[b + 1][:, :, t0:t0 + tw].rearrange(
                                    "k p t -> p k t"),
                            )
                tp = tps[b]
                for (t0, tw) in chunks:
                    # conv stage 1: attc[c,t]; row 10 = 1.0 (ones channel)
                    attc_ps = sp.tile([NC11, 512], mybir.dt.float32, tag="attc",
                                      name="attc_ps")
                    for s_ in range(2):
                        nc.tensor.matmul(
                            attc_ps[:, :tw],
                            convwT[:, s_, 0:NC11],
                            tp[:, s_, t0:t0 + tw],
                            start=(s_ == 0), stop=(s_ == 1),
                        )
                    attc4 = workp.tile([NC11, 512], dt_mm, tag="attc4",
                                       name="attc4")
                    nc.vector.tensor_copy(attc4[:, :tw], attc_ps[:, :tw])

                    # z = pre_enc (+ att2 + bias via packed K=11 matmuls)
                    zps = [
                        zp.tile([128, 2, 512], mybir.dt.float32, tag="z",
                                name=f"zps{_i}")
                        for _i in range(2)
                    ]
                    for m in range(NMT):
                        for kp in range(NKP):
                            nc.tensor.matmul(
                                zps[m // 2][:, m % 2, :tw],
                                wencT[:, kp, m * 128:(m + 1) * 128],
                                enc_tiles[b][:, kp, t0:t0 + tw],
                                start=(kp == 0), stop=False,
                            )
                    for m in range(NMT):
                        nc.tensor.matmul(
                            zps[m // 2][:, m % 2, :tw],
                            watt4[b][:, m * 128:(m + 1) * 128],
                            attc4[:, :tw],
                            start=False, stop=True,
                        )
                    tanhz = tanhp.tile([128, 2, 2, 512], dt_mm, tag="tanhz",
                                       name="tanhz")
                    for pair in range(2):
                        nc.scalar.activation(
                            tanhz[:, pair, :, :tw], zps[pair][:, :, :tw],
                            mybir.ActivationFunctionType.Tanh,
                        )
                    # e[t] = gvec . tanhz
                    e_ps = ep.tile([1, 512], mybir.dt.float32, tag="e",
                                   name="e_ps")
                    for m in range(NMT):
                        nc.tensor.matmul(
                            e_ps[:, :tw],
                            gvec[:, m:m + 1],
                            tanhz[:, m // 2, m % 2, :tw],
                            start=(m == 0), stop=(m == NMT - 1),
                        )
                    nc.vector.tensor_copy(
                        e_stage[0:1, b * T + t0:b * T + t0 + tw], e_ps[:, :tw]
                    )
                    ci = CHUNKS.index((t0, tw))
                    nc.vector.reduce_max(
                        emax4[0:1, b * len(CHUNKS) + ci:b * len(CHUNKS) + ci + 1],
                        e_stage[0:1, b * T + t0:b * T + t0 + tw],
                        axis=mybir.AxisListType.X,
                    )

            def emit_softmax(b):
                # unnormalized softmax; rinv rides the broadcast matmul
                erow = e_stage[0:1, b * T:(b + 1) * T]
                emax = emax_all[0:1, b:b + 1]
                nc.vector.reduce_max(
                    emax,
                    emax4[0:1, b * len(CHUNKS):(b + 1) * len(CHUNKS)],
                    axis=mybir.AxisListType.X,
                )
                nb2 = nb2_all[0:1, b:b + 1]
                nc.vector.tensor_scalar_mul(nb2, emax, -SCALING)
                esum = esum_all[0:1, b:b + 1]
                nc.scalar.activation(
                    wexp_st[0:1, b * T:(b + 1) * T], erow,
                    mybir.ActivationFunctionType.Exp,
                    bias=nb2, scale=SCALING, accum_out=esum,
                )
                rinv = rinv_all[0:1, b:b + 1]
                nc.vector.reciprocal(rinv, esum)
                wnorm = smallp.tile([1, T], dt_mm, tag="wnorm", name="wnorm")
                nc.vector.tensor_scalar_mul(
                    wnorm[:], wexp_st[0:1, b * T:(b + 1) * T], rinv
                )
                nc.sync.dma_start(w128[32 * b:32 * b + 1, 0:T], wnorm[:])

            def emit_context(b):
                # c[d] = sum_t w[t] enc[t, d] on the PE: transpose w into
                # t-on-partitions via the DMA XBAR, then 13 accumulating
                # matvecs against the natural-layout enc tiles
                wT = workp.tile([128, 13, 32], dt_mm, tag="wT", name="wT")
                for ci in range(13):
                    nc.sync.dma_start(
                        wT[:, ci, :],
                        w128[32 * b:32 * (b + 1), ci * 128:(ci + 1) * 128],
                        transpose=True,
                    )
                c_ps = ep.tile([1, 512], mybir.dt.float32, tag="e",
                               name="c_ps")
                for ci in range(13):
                    nc.tensor.matmul(
                        c_ps[:],
                        wT[:, ci, 0:1],
                        encn_tiles[b][:, ci, :],
                        start=(ci == 0), stop=(ci == 12),
                    )
                c_row = smallp.tile([1, ATT_DIM], f32, tag="crow",
                                    name="c_row")
                nc.scalar.copy(c_row[:], c_ps[:])
                nc.sync.dma_start(c_out_d[b:b + 1, :], c_row[:])

            def emit_all():
                # software pipeline: context(b-1) emitted after mains(b) so
                # the PE never stalls waiting for batch b-1's softmax
                for _b in range(NB):
                    tps[_b] = None
                nc.vector.memset(w128[:, T:], 0.0)
                for (t0, tw) in CHUNKS:
                    nc.sync.dma_start(
                        enc_tiles[0][:, :, t0:t0 + tw],
                        encT_d[0][:, :, t0:t0 + tw].rearrange("k p t -> p k t"),
                    )
                wexp4 = smallp.tile([NB, T], dt_mm, name="wexp4")
                rinv4 = smallp.tile([NB, 1], f32, name="rinv4")
                emit_dec()
                for b in range(NB):
                    emit_mains(b, CHUNKS)
                    if b > 0:
                        emit_context(b - 1)
                    emit_softmax(b)
                    nc.sync.dma_start(
                        wexp4[b:b + 1, :], wexp_st[0:1, b * T:(b + 1) * T]
                    )
                    nc.sync.dma_start(rinv4[b:b + 1, :], rinv_all[0:1, b:b + 1])
                w4 = smallp.tile([NB, T], f32, name="w4")
                nc.vector.tensor_scalar_mul(w4[:], wexp4[:], rinv4[:])
                nc.sync.dma_start(w_out_d.rearrange("(b t) -> b t", b=NB), w4[:])
                emit_context(NB - 1)

            if _LOOP > 1:
                with tc.For_i(0, _LOOP, 1):
                    emit_all()
            else:
                emit_all()

    nc.compile()
    return nc


def _host_prep(enc_hs_pad, dec_z, att_prev, W_enc, b_enc, W_dec, W_att,
               conv_w, gvec_w):
    npdt = _np_dt()
    f32 = np.float32

    encT = np.ascontiguousarray(
        enc_hs_pad.astype(f32).transpose(0, 2, 1)
    ).reshape(B, NKP, 128, T).astype(npdt)

    encN = np.zeros((B, 13 * 128, ATT_DIM), f32)
    encN[:, :T] = enc_hs_pad.astype(f32)
    encN = encN.reshape(B, 13, 128, ATT_DIM).astype(npdt)

    padded = np.zeros((B, T + 2 * ACONV_FILTS), f32)
    padded[:, ACONV_FILTS:ACONV_FILTS + T] = att_prev
    win = np.lib.stride_tricks.sliding_window_view(padded, T, axis=1)  # [B,201,T]
    toep = np.zeros((B, 256, T), f32)
    toep[:, :KSIZE] = win
    toep[:, KSIZE] = 1.0  # ones row -> attc ones channel
    toep = toep.reshape(B, 2, 128, T).astype(npdt)

    wencT = np.ascontiguousarray(W_enc.astype(f32).T).reshape(NKP, 128, ATT_DIM).astype(npdt)

    # stage2 stationary: rows 0-9 = W_att.T; row 10 = dec bias (device patch)
    watt4 = np.zeros((ACONV_CHANS + 1, ATT_DIM), f32)
    watt4[:ACONV_CHANS] = W_att.astype(f32).T
    watt4 = watt4.astype(npdt)

    cw = conv_w.reshape(ACONV_CHANS, KSIZE).astype(f32).T  # [201, 10]
    convwT = np.zeros((256, 32), f32)
    convwT[:KSIZE, :ACONV_CHANS] = cw
    convwT[KSIZE, ACONV_CHANS] = 1.0  # ones channel reads the Toeplitz ones row
    convwT = convwT.reshape(2, 128, 32).astype(npdt)

    wdecT = np.ascontiguousarray(W_dec.astype(f32).T).reshape(NKD, 128, ATT_DIM).astype(npdt)
    deczT = np.ascontiguousarray(dec_z.astype(f32).T).reshape(NKD, 128, B).astype(npdt)
    bencT = np.ascontiguousarray(b_enc.astype(f32).reshape(1, ATT_DIM)).astype(npdt)
    gvec = np.ascontiguousarray(gvec_w.astype(f32).reshape(NMT, 128).T).astype(npdt)
    ident = np.eye(128, dtype=f32).astype(npdt)

    in_maps = []
    for i in range(NCORES):
        sl = slice(i * NB, (i + 1) * NB)
        in_maps.append({
            "encT": encT[sl],
            "encN": encN[sl],
            "toep": toep[sl],
            "wencT": wencT,
            "watt4": watt4,
            "convwT": convwT,
            "wdecT": wdecT,
            "deczT": np.ascontiguousarray(deczT[:, :, sl]),
            "bencT": bencT,
            "gvec": gvec,
            "ident": ident,
        })
    return in_maps


def kernel(enc_hs_pad, enc_hs_len, dec_z, att_prev, W_enc, b_enc, W_dec,
           W_att, conv_w, gvec_w, gvec_b):
    from concourse.bass_utils import run_bass_kernel_spmd

    if "nc" not in _cache:
        _cache["nc"] = _build_nc()
    nc = _cache["nc"]

    in_maps = _host_prep(
        np.asarray(enc_hs_pad), np.asarray(dec_z), np.asarray(att_prev),
        np.asarray(W_enc), np.asarray(b_enc), np.asarray(W_dec),
        np.asarray(W_att), np.asarray(conv_w), np.asarray(gvec_w),
    )

    trace = os.environ.get("KERNEL_TRACE", "0") == "1"
    res = run_bass_kernel_spmd(
        nc, in_maps, core_ids=list(range(NCORES)), trace=trace,
    )
    _cache["last_result"] = res

    c_full = np.zeros((B, ATT_DIM), np.float32)
    w_full = np.zeros((B, T), np.float32)
    for i in range(NCORES):
        out = res.results[i]
        c_full[i * NB:(i + 1) * NB] = out["c_out"]
        w_full[i * NB:(i + 1) * NB] = out["w_out"].reshape(NB, T)
    return c_full, w_full
